# revision 35
# baseline (speedup 1.0000x reference)
"""Trainium2 Bass kernel for FBPINN-with-window (dense MoE over 16 subnets).

Math (per point n):
    h   = relu(x @ pW0 + pb0); h += relu(h @ pWmid_l + pbmid_l) (x2)
    z   = h @ pWl + pbl;  ez = exp(z)            (softmax un-normalized)
    xn_c = (x - center_c)/scale_c  (folded on host into layer-0 weights)
    g_c = tanh(xn_c @ W0_c + b0_c); g_c = tanh(g_c @ Wmid_cl + bmid_cl) (x2)
    u_c = g_c @ Wl_c + bl_c
    acc = sum_c softmax(z)_c * u_c = (sum_c ez_c*(g_c@Wl_c) + sum_c ez_c*bl_c) / sum_c ez_c
    out = acc * x0(1-x0)*x1(1-x1)

Device layout: activations transposed (features on partitions, points on the
free dim).  Data-parallel over 8 cores (8192 points each), 4 point-tiles of
2048 per core.  The ACT (scalar) engine's tanh throughput is the roofline
(~96 x [128,2048] tanh per tile); the design goal is keeping ACT ~100% busy:
  - all subnet weights resident in SBUF (bf16 for mids/last), no streaming
  - two subnet chains in flight, staggered by 3 steps, so one chain's
    matmuls/psum drains overlap the other chain's tanh
  - per-chain last-layer output (PSUM) drained by DVE + DMA off ACT's path
  - x staged in DRAM pre-replicated at rows 32r (one contiguous DMA per
    point-tile); prologue DMAs split between the sync queue (critical path:
    tile-0 x, first subnets' weights) and the gpsimd queue (bulk weights)
"""

import os

import numpy as np

N = 65536
D = 2
C = 16
PH = 128
PNMID = 2
SW = 256
SNMID = 2

NCORES = 8
NP = N // NCORES          # 8192 points per core
PTILE = 2048              # points per tile
NT = NP // PTILE          # 4 tiles
CHK = 512                 # matmul moving free dim (one PSUM bank)
NCH = PTILE // CHK        # 4 chunks per tile
FC = SW // 128            # 2 feature chunks
KC = SW // 128            # 2 contraction chunks
PPB = NP // 128           # 64 points per partition (points-layout)
WBLK = SNMID * FC * KC * 128  # mid-weight cols per subnet (1024)

STAG = int(os.environ.get("KERNEL_STAG", "3"))  # lane-B stagger in steps

_CACHE = {}


def _build():
    import concourse.mybir as mybir
    import concourse.tile as tile
    from concourse import bacc

    f32 = mybir.dt.float32
    f32r = mybir.dt.float32r
    bf16 = mybir.dt.bfloat16
    AF = mybir.ActivationFunctionType
    OP = mybir.AluOpType

    nc = bacc.Bacc("TRN2", debug=False)

    def din(name, shape, dt=f32):
        return nc.dram_tensor(name, shape, dt, kind="ExternalInput").ap()

    xT = din("xT", (128, NP), bf16)   # [x0;x1;1] replicated at rows 32r
    xP = din("xP", (128, 2 * PPB))
    w0q = din("w0q", (128, C * FC * 128), bf16)
    wm = din("wm", (128, C * WBLK), bf16)
    bm = din("bm", (128, C * SNMID * FC))
    wl = din("wl", (128, C * KC), bf16)
    pw0q = din("pw0q", (128, PH), bf16)
    pwm = din("pwm", (PH, PNMID * PH), f32r)
    pbm = din("pbm", (PH, PNMID))
    pwl = din("pwl", (PH, C), f32r)
    pbl = din("pbl", (C, 1))
    cw = din("cw", (C, 2), f32r)
    y = nc.dram_tensor("y", (NP,), f32, kind="ExternalOutput").ap()

    with tile.TileContext(nc) as tc:
        with (
            tc.tile_pool(name="wp", bufs=1) as wp,
            tc.tile_pool(name="gp", bufs=10) as gp,
            tc.tile_pool(name="hp", bufs=2) as hp,
            tc.tile_pool(name="sp", bufs=2) as sp,
            tc.tile_pool(name="rp", bufs=2) as rp,
            tc.tile_pool(name="xp", bufs=2) as xpl,
            tc.tile_pool(name="fin", bufs=1) as fin,
            tc.tile_pool(name="pp", bufs=2, space="PSUM") as pp,
        ):
            # ---- critical-path DMAs on sync: tile-0 x, c0-3 weights ----
            xt0 = xpl.tile([128, PTILE], bf16, tag="xt")
            nc.sync.dma_start(xt0[:], xT[:, 0:PTILE])
            s_w0q = wp.tile([128, C * FC * 128], bf16)
            qb = C * FC * 32  # 1024 cols = 4 subnets worth
            nc.sync.dma_start(s_w0q[:, 0:qb], w0q[:, 0:qb])
            s_pw0q = wp.tile([128, PH], bf16)
            nc.sync.dma_start(s_pw0q[:], pw0q)
            s_wm = wp.tile([128, C * WBLK], bf16)
            wb = C * WBLK // 8  # 2048 cols = 2 subnets worth
            nc.sync.dma_start(s_wm[:, 0:wb], wm[:, 0:wb])
            s_bm = wp.tile([128, C * SNMID * FC], f32)
            nc.sync.dma_start(s_bm[:], bm)
            s_pwm = wp.tile([PH, PNMID * PH], f32r)
            nc.sync.dma_start(s_pwm[:], pwm)
            # ---- bulk weights on the (idle) gpsimd queue ----
            s_pbm = wp.tile([PH, PNMID], f32)
            nc.gpsimd.dma_start(s_pbm[:], pbm)
            s_pwl = wp.tile([PH, C], f32r)
            nc.gpsimd.dma_start(s_pwl[:], pwl)
            s_pbl = wp.tile([C, 1], f32)
            nc.gpsimd.dma_start(s_pbl[:], pbl)
            for cq in range(1, 4):
                nc.gpsimd.dma_start(
                    s_w0q[:, cq * qb : (cq + 1) * qb], w0q[:, cq * qb : (cq + 1) * qb]
                )
            s_wl = wp.tile([128, C * KC], bf16)
            nc.gpsimd.dma_start(s_wl[:], wl)
            s_cw = wp.tile([C, 2], f32r)
            nc.gpsimd.dma_start(s_cw[:], cw)
            for cq in range(1, 8):
                nc.gpsimd.dma_start(
                    s_wm[:, cq * wb : (cq + 1) * wb], wm[:, cq * wb : (cq + 1) * wb]
                )

            # ---- per-core x (points-layout) + boundary factor ----
            s_xP = fin.tile([128, 2 * PPB], f32)
            nc.gpsimd.dma_start(s_xP[:], xP)
            s_xmx = fin.tile([128, 2 * PPB], f32)
            nc.vector.tensor_mul(s_xmx[:], s_xP[:], s_xP[:])
            nc.vector.tensor_sub(s_xmx[:], s_xP[:], s_xmx[:])
            v = s_xmx.rearrange("p (j two) -> p j two", two=2)
            s_bc = fin.tile([128, PPB], f32)
            nc.vector.tensor_mul(s_bc[:], v[:, :, 0], v[:, :, 1])

            # points-layout accumulators, filled per tile via reshape DMAs
            s_accP = fin.tile([128, PPB], f32)
            s_s1P = fin.tile([128, PPB], f32)
            s_s2P = fin.tile([128, PPB], f32)

            xt4_next = {}
            pou_pst = {}
            pending = [None]
            for t in range(NT):
                toff = t * PTILE
                xt4 = xt0 if t == 0 else xt4_next.pop(0)

                pst = pou_pst.setdefault(t, {})

                def make_pou(tt, xt_tt, pstd):
                    def pou_l0():
                        ps0 = pp.tile([PH, PTILE], f32, tag="mm")
                        for rr in range(NCH):
                            nc.tensor.matmul(
                                ps0[:, rr * CHK : (rr + 1) * CHK],
                                s_pw0q[32 * rr : 32 * rr + 3, :],
                                xt_tt[32 * rr : 32 * rr + 3, rr * CHK : (rr + 1) * CHK],
                                start=True,
                                stop=True,
                                tile_position=(32 * rr, 0),
                            )
                        h = hp.tile([PH, PTILE], f32r, tag="h")
                        nc.vector.tensor_scalar_max(h[:], ps0[:], 0.0)
                        pstd["h"] = h

                    def pou_mid(l):
                        h = pstd["h"]
                        psl = pp.tile([PH, PTILE], f32, tag="mm")
                        for n in range(NCH):
                            nc.tensor.matmul(
                                psl[:, n * CHK : (n + 1) * CHK],
                                s_pwm[:, l * PH : (l + 1) * PH],
                                h[:, n * CHK : (n + 1) * CHK],
                                start=True,
                                stop=True,
                            )
                        hr = hp.tile([PH, PTILE], f32r, tag="h")
                        nc.vector.tensor_scalar(
                            hr[:], psl[:], s_pbm[:, l : l + 1], 0.0, op0=OP.add, op1=OP.max
                        )
                        nc.vector.tensor_add(hr[:], hr[:], h[:])
                        pstd["h"] = hr

                    def pou_logits():
                        h = pstd["h"]
                        psz = pp.tile([C, PTILE], f32, tag="mm")
                        for n in range(NCH):
                            nc.tensor.matmul(
                                psz[:, n * CHK : (n + 1) * CHK],
                                s_pwl[:],
                                h[:, n * CHK : (n + 1) * CHK],
                                start=True,
                                stop=True,
                            )
                        ez = sp.tile([C, PTILE], f32r, tag="ez")
                        nc.scalar.activation(ez[:], psz[:], AF.Exp, bias=s_pbl[:, 0:1])
                        pstd["ez"] = ez

                    def pou_s12():
                        ez = pstd["ez"]
                        pss = pp.tile([2, PTILE], f32, tag="mm")
                        for n in range(NCH):
                            nc.tensor.matmul(
                                pss[:, n * CHK : (n + 1) * CHK],
                                s_cw[:],
                                ez[:, n * CHK : (n + 1) * CHK],
                                start=True,
                                stop=True,
                            )
                        s12row = rp.tile([2, PTILE], f32, tag="row")
                        nc.vector.tensor_copy(s12row[:], pss[:])
                        nc.sync.dma_start(
                            s_s1P[tt * 32 : (tt + 1) * 32, :], s12row[0:1, :]
                        )
                        nc.sync.dma_start(
                            s_s2P[tt * 32 : (tt + 1) * 32, :], s12row[1:2, :]
                        )

                    return [
                        pou_l0,
                        lambda: pou_mid(0),
                        lambda: pou_mid(1),
                        pou_logits,
                        pou_s12,
                    ]

                pou_steps = make_pou(t, xt4, pst) if t == 0 else None

                # ---------- subnets (two staggered lanes) ----------
                u_asm = sp.tile([C, PTILE], f32r, tag="ua")

                def sub_l0(c, st, fc):
                    if fc == 0:
                        st["g0"] = gp.tile([128, KC, PTILE], bf16, tag="g", name="g0")
                    col = (c * FC + fc) * 128
                    pt = pp.tile([128, PTILE], f32, tag="mm")
                    for rr in range(NCH):
                        nc.tensor.matmul(
                            pt[:, rr * CHK : (rr + 1) * CHK],
                            s_w0q[32 * rr : 32 * rr + 3, col : col + 128],
                            xt4[32 * rr : 32 * rr + 3, rr * CHK : (rr + 1) * CHK],
                            start=True,
                            stop=True,
                            tile_position=(32 * rr, 0),
                        )
                    nc.scalar.activation(st["g0"][:, fc, :], pt[:], AF.Tanh)

                def sub_mid(c, st, l, fc):
                    gcur = st[f"g{l}"]
                    if fc == 0:
                        st[f"g{l + 1}"] = gp.tile(
                            [128, KC, PTILE], bf16, tag="g", name=f"g{l + 1}"
                        )
                    pt = pp.tile([128, PTILE], f32, tag="mm")
                    for kc in range(KC):
                        col = c * WBLK + ((l * FC + fc) * KC + kc) * 128
                        for n in range(NCH):
                            nc.tensor.matmul(
                                pt[:, n * CHK : (n + 1) * CHK],
                                s_wm[:, col : col + 128],
                                gcur[:, kc, n * CHK : (n + 1) * CHK],
                                start=(kc == 0),
                                stop=(kc == KC - 1),
                            )
                    bcol = (c * SNMID + l) * FC + fc
                    nc.scalar.activation(
                        st[f"g{l + 1}"][:, fc, :],
                        pt[:],
                        AF.Tanh,
                        bias=s_bm[:, bcol : bcol + 1],
                    )

                def pod_last(p):
                    # last layer for subnets 4p..4p+3, one [1,512] output per
                    # (subnet, chunk) packed into 4 concurrent PE col groups
                    pu = pp.tile([128, PTILE], f32, tag="mm")
                    for kc in range(KC):
                        for n in range(NCH):
                            for j in range(4):
                                c = 4 * p + j
                                gcur = states[c][f"g{SNMID}"]
                                nc.tensor.matmul(
                                    pu[32 * j : 32 * j + 1, n * CHK : (n + 1) * CHK],
                                    s_wl[:, c * KC + kc : c * KC + kc + 1],
                                    gcur[:, kc, n * CHK : (n + 1) * CHK],
                                    start=(kc == 0),
                                    stop=(kc == KC - 1),
                                    tile_position=(0, 32 * j),
                                )
                    urows = rp.tile([128, PTILE], f32r, tag="row")
                    nc.vector.tensor_copy(urows[0:97, :], pu[0:97, :])
                    for j in range(4):
                        nc.sync.dma_start(
                            u_asm[4 * p + j : 4 * p + j + 1, :],
                            urows[32 * j : 32 * j + 1, :],
                        )

                def sub_step(c, st, k):
                    if k <= 1:
                        sub_l0(c, st, k)
                    else:
                        sub_mid(c, st, (k - 2) // FC, (k - 2) % FC)

                # lane A: even subnets, lane B: odd subnets (B lags by STAG);
                # chains run 6 steps (l0 x2, mid x4); last layer runs as pods
                # of 4 subnets packed into PE col quadrants
                stepsA = [(c, k) for c in range(0, C, 2) for k in range(6)]
                stepsB = [(c, k) for c in range(1, C, 2) for k in range(6)]
                states = [dict() for _ in range(C)]
                nslots = len(stepsA) + STAG
                pou_next = [None]
                for s in range(nslots):
                    if t == 0 and s < 5:
                        pou_steps[s]()
                    if s == 4 and pending[0] is not None:
                        pending[0]()
                        pending[0] = None
                    if s == 14 and t + 1 < NT:
                        nxt = xpl.tile([128, PTILE], bf16, tag="xt", name="xt4")
                        nc.sync.dma_start(
                            nxt[:], xT[:, (t + 1) * PTILE : (t + 2) * PTILE]
                        )
                        xt4_next[0] = nxt
                        pou_next[0] = make_pou(
                            t + 1, nxt, pou_pst.setdefault(t + 1, {})
                        )
                    if s in (16, 22, 30, 36, 44) and pou_next[0] is not None:
                        pou_next[0][(16, 22, 30, 36, 44).index(s)]()
                    if s < len(stepsA):
                        c, k = stepsA[s]
                        sub_step(c, states[c], k)
                    if s >= STAG:
                        c, k = stepsB[s - STAG]
                        sub_step(c, states[c], k)
                    if s in (15, 27, 39):
                        pod_last((s - 15) // 12)
                    if s == 50:
                        pod_last(3)

                # ---------- windowed combine (deferred into next tile) ----------
                def make_combine(t, u_asm, ez):
                    def combine():
                        # DVE mul on the last tile: it sits on the serial tail
                        if t == NT - 1:
                            nc.vector.tensor_mul(u_asm[:], u_asm[:], ez[:])
                        else:
                            nc.gpsimd.tensor_mul(u_asm[:], u_asm[:], ez[:])
                        pacc = pp.tile([1, PTILE], f32, tag="mm")
                        for n in range(NCH):
                            nc.tensor.matmul(
                                pacc[:, n * CHK : (n + 1) * CHK],
                                s_cw[:, 0:1],
                                u_asm[:, n * CHK : (n + 1) * CHK],
                                start=True,
                                stop=True,
                            )
                        accrow = rp.tile([2, PTILE], f32, tag="row")
                        nc.vector.tensor_copy(accrow[0:1, :], pacc[:])
                        nc.sync.dma_start(
                            s_accP[t * 32 : (t + 1) * 32, :], accrow[0:1, :]
                        )

                    return combine

                pending[0] = make_combine(t, u_asm, pst["ez"])

            if pending[0] is not None:
                pending[0]()
                pending[0] = None

            # ---------- final: combine in points-layout ----------
            s_r = fin.tile([128, PPB], f32)
            nc.vector.reciprocal(s_r[:], s_s1P[:])
            s_num = fin.tile([128, PPB], f32)
            nc.vector.tensor_add(s_num[:], s_accP[:], s_s2P[:])
            nc.vector.tensor_mul(s_num[:], s_num[:], s_r[:])
            nc.vector.tensor_mul(s_num[:], s_num[:], s_bc[:])
            nc.sync.dma_start(y.rearrange("(p j) -> p j", p=128), s_num[:])

    nc.compile()
    return nc


def _prep_inputs(inputs):
    import ml_dtypes

    f = lambda k: np.ascontiguousarray(np.asarray(inputs[k]), dtype=np.float32)
    x = f("x")
    centers, scales = f("centers"), f("scales")
    sub_W0, sub_b0 = f("sub_W0"), f("sub_b0")
    sub_Wmid, sub_bmid = f("sub_Wmid"), f("sub_bmid")
    sub_Wl, sub_bl = f("sub_Wl"), f("sub_bl")

    # fold per-subdomain normalization into layer-0 weights:
    # xn = (x - c)/s  =>  xn @ W0 + b0 = x @ (W0/s) + (b0 - (c/s) @ W0)
    w0e_full = sub_W0 / scales[:, :, None]                       # [C, D, SW]
    b0e_full = sub_b0 - np.einsum("cd,cdw->cw", centers / scales, sub_W0)

    # row-group packed layer-0 weights: rows {32r,32r+1,32r+2} = [W0; W1; b]
    w0q = np.zeros((128, C * FC * 128), np.float32)
    for c in range(C):
        for fc in range(FC):
            col = (c * FC + fc) * 128
            blk = np.vstack(
                [
                    w0e_full[c][:, fc * 128 : (fc + 1) * 128],
                    b0e_full[c][None, fc * 128 : (fc + 1) * 128],
                ]
            )
            for rr in range(4):
                w0q[32 * rr : 32 * rr + 3, col : col + 128] = blk

    middt_np = ml_dtypes.bfloat16
    wm = np.ascontiguousarray(
        sub_Wmid.reshape(C, SNMID, KC, 128, FC, 128)
        .transpose(3, 0, 1, 4, 2, 5)
        .reshape(128, C * WBLK)
    ).astype(middt_np)
    bm = np.ascontiguousarray(
        sub_bmid.reshape(C, SNMID, FC, 128).transpose(3, 0, 1, 2).reshape(128, -1)
    )
    wl = np.ascontiguousarray(
        sub_Wl.reshape(C, KC, 128).transpose(2, 0, 1).reshape(128, -1)
    ).astype(middt_np)
    cwm = np.ascontiguousarray(
        np.stack([np.ones(C, np.float32), sub_bl[:, 0]], axis=1)
    )

    pw0q = np.zeros((128, PH), np.float32)
    pblk = np.vstack([f("pou_W0"), f("pou_b0")[None, :]])
    for rr in range(4):
        pw0q[32 * rr : 32 * rr + 3, :] = pblk

    shared = dict(
        w0q=w0q.astype(middt_np),
        wm=wm,
        bm=bm,
        wl=wl,
        pw0q=pw0q.astype(middt_np),
        pwm=np.ascontiguousarray(f("pou_Wmid").transpose(1, 0, 2).reshape(PH, -1)),
        pbm=np.ascontiguousarray(f("pou_bmid").T),
        pwl=f("pou_Wl"),
        pbl=np.ascontiguousarray(f("pou_bl")[:, None]),
        cw=cwm,
    )

    in_maps = []
    for core in range(NCORES):
        xs = x[core * NP : (core + 1) * NP]
        m = dict(shared)
        xt3 = np.vstack([xs.T, np.ones((1, NP), np.float32)])  # [3, NP]
        xtq = np.zeros((128, NP), np.float32)
        for rr in range(4):
            xtq[32 * rr : 32 * rr + 3, :] = xt3
        m["xT"] = np.ascontiguousarray(xtq).astype(ml_dtypes.bfloat16)
        m["xP"] = np.ascontiguousarray(xs.reshape(128, 2 * PPB))
        in_maps.append(m)
    return in_maps


def kernel(**inputs):
    from concourse.bass_utils import run_bass_kernel_spmd

    if "nc" not in _CACHE:
        _CACHE["nc"] = _build()
    nc = _CACHE["nc"]

    in_maps = _prep_inputs(inputs)
    trace = os.environ.get("KERNEL_TRACE", "0") == "1"
    res = run_bass_kernel_spmd(
        nc, in_maps, core_ids=list(range(NCORES)), trace=trace
    )
    kernel.last_results = res
    y = np.concatenate([res.results[i]["y"] for i in range(NCORES)])
    return y.astype(np.float32)


# revision 41
# speedup vs baseline: 1.0068x; 1.0068x over previous
"""Trainium2 Bass kernel for FBPINN-with-window (dense MoE over 16 subnets).

Math (per point n):
    h   = relu(x @ pW0 + pb0); h += relu(h @ pWmid_l + pbmid_l) (x2)
    z   = h @ pWl + pbl;  ez = exp(z)            (softmax un-normalized)
    xn_c = (x - center_c)/scale_c  (folded on host into layer-0 weights)
    g_c = tanh(xn_c @ W0_c + b0_c); g_c = tanh(g_c @ Wmid_cl + bmid_cl) (x2)
    u_c = g_c @ Wl_c + bl_c
    acc = sum_c softmax(z)_c * u_c = (sum_c ez_c*(g_c@Wl_c) + sum_c ez_c*bl_c) / sum_c ez_c
    out = acc * x0(1-x0)*x1(1-x1)

Device layout: activations transposed (features on partitions, points on the
free dim).  Data-parallel over 8 cores (8192 points each), 4 point-tiles of
2048 per core.  The ACT (scalar) engine's tanh throughput is the roofline
(~96 x [128,2048] tanh per tile); the design goal is keeping ACT ~100% busy:
  - all subnet weights resident in SBUF (bf16 for mids/last), no streaming
  - two subnet chains in flight, staggered by 3 steps, so one chain's
    matmuls/psum drains overlap the other chain's tanh
  - per-chain last-layer output (PSUM) drained by DVE + DMA off ACT's path
  - x staged in DRAM pre-replicated at rows 32r (one contiguous DMA per
    point-tile); prologue DMAs split between the sync queue (critical path:
    tile-0 x, first subnets' weights) and the gpsimd queue (bulk weights)
"""

import os

import numpy as np

N = 65536
D = 2
C = 16
PH = 128
PNMID = 2
SW = 256
SNMID = 2

NCORES = 8
NP = N // NCORES          # 8192 points per core
PTILE = 2048              # points per tile
NT = NP // PTILE          # 4 tiles
CHK = 512                 # matmul moving free dim (one PSUM bank)
NCH = PTILE // CHK        # 4 chunks per tile
FC = SW // 128            # 2 feature chunks
KC = SW // 128            # 2 contraction chunks
PPB = NP // 128           # 64 points per partition (points-layout)
WBLK = SNMID * FC * KC * 128  # mid-weight cols per subnet (1024)

STAG = int(os.environ.get("KERNEL_STAG", "3"))  # lane-B stagger in steps

_CACHE = {}


def _build():
    import concourse.mybir as mybir
    import concourse.tile as tile
    from concourse import bacc

    f32 = mybir.dt.float32
    f32r = mybir.dt.float32r
    bf16 = mybir.dt.bfloat16
    AF = mybir.ActivationFunctionType
    OP = mybir.AluOpType

    nc = bacc.Bacc("TRN2", debug=False)

    def din(name, shape, dt=f32):
        return nc.dram_tensor(name, shape, dt, kind="ExternalInput").ap()

    xT = din("xT", (128, NP), bf16)   # [x0;x1;1] replicated at rows 32r
    xP = din("xP", (128, 2 * PPB))
    w0q = din("w0q", (128, C * FC * 128), bf16)
    wm = din("wm", (128, C * WBLK), bf16)
    bm = din("bm", (128, C * SNMID * FC))
    wl = din("wl", (128, C * KC), bf16)
    pw0q = din("pw0q", (128, PH), bf16)
    pwm = din("pwm", (PH, PNMID * PH), f32r)
    pbm = din("pbm", (PH, PNMID))
    pwl = din("pwl", (PH, C), f32r)
    pbl = din("pbl", (C, 1))
    cw = din("cw", (C, 2), f32r)
    y = nc.dram_tensor("y", (NP,), f32, kind="ExternalOutput").ap()

    with tile.TileContext(nc) as tc:
        with (
            tc.tile_pool(name="wp", bufs=1) as wp,
            tc.tile_pool(name="gp", bufs=10) as gp,
            tc.tile_pool(name="hp", bufs=2) as hp,
            tc.tile_pool(name="sp", bufs=2) as sp,
            tc.tile_pool(name="rp", bufs=2) as rp,
            tc.tile_pool(name="xp", bufs=2) as xpl,
            tc.tile_pool(name="fin", bufs=1) as fin,
            tc.tile_pool(name="pp", bufs=2, space="PSUM") as pp,
        ):
            # ---- critical-path DMAs fanned across idle engine queues ----
            xt0 = xpl.tile([128, PTILE], bf16, tag="xt")
            nc.sync.dma_start(xt0[:], xT[:, 0:PTILE])
            s_w0q = wp.tile([128, C * FC * 128], bf16)
            qb = C * FC * 32  # 1024 cols = 4 subnets worth
            nc.scalar.dma_start(s_w0q[:, 0:qb], w0q[:, 0:qb])
            s_pw0q = wp.tile([128, PH], bf16)
            nc.scalar.dma_start(s_pw0q[:], pw0q)
            s_wm = wp.tile([128, C * WBLK], bf16)
            wb = C * WBLK // 8  # 2048 cols = 2 subnets worth
            nc.gpsimd.dma_start(s_wm[:, 0:wb], wm[:, 0:wb])
            s_bm = wp.tile([128, C * SNMID * FC], f32)
            nc.sync.dma_start(s_bm[:], bm)
            s_pwm = wp.tile([PH, PNMID * PH], f32r)
            nc.sync.dma_start(s_pwm[:], pwm)
            # ---- bulk weights on the (idle) gpsimd queue ----
            s_pbm = wp.tile([PH, PNMID], f32)
            nc.gpsimd.dma_start(s_pbm[:], pbm)
            s_pwl = wp.tile([PH, C], f32r)
            nc.gpsimd.dma_start(s_pwl[:], pwl)
            s_pbl = wp.tile([C, 1], f32)
            nc.gpsimd.dma_start(s_pbl[:], pbl)
            for cq in range(1, 4):
                nc.gpsimd.dma_start(
                    s_w0q[:, cq * qb : (cq + 1) * qb], w0q[:, cq * qb : (cq + 1) * qb]
                )
            s_wl = wp.tile([128, C * KC], bf16)
            nc.gpsimd.dma_start(s_wl[:], wl)
            s_cw = wp.tile([C, 2], f32r)
            nc.gpsimd.dma_start(s_cw[:], cw)
            for cq in range(1, 8):
                nc.gpsimd.dma_start(
                    s_wm[:, cq * wb : (cq + 1) * wb], wm[:, cq * wb : (cq + 1) * wb]
                )

            # ---- per-core x (points-layout) + boundary factor ----
            s_xP = fin.tile([128, 2 * PPB], f32)
            nc.gpsimd.dma_start(s_xP[:], xP)
            s_xmx = fin.tile([128, 2 * PPB], f32)
            nc.vector.tensor_mul(s_xmx[:], s_xP[:], s_xP[:])
            nc.vector.tensor_sub(s_xmx[:], s_xP[:], s_xmx[:])
            v = s_xmx.rearrange("p (j two) -> p j two", two=2)
            s_bc = fin.tile([128, PPB], f32)
            nc.vector.tensor_mul(s_bc[:], v[:, :, 0], v[:, :, 1])

            # points-layout accumulators, filled per tile via reshape DMAs
            s_accP = fin.tile([128, PPB], f32)
            s_s1P = fin.tile([128, PPB], f32)
            s_s2P = fin.tile([128, PPB], f32)

            xt4_next = {}
            pou_pst = {}
            pending = [None]
            for t in range(NT):
                toff = t * PTILE
                xt4 = xt0 if t == 0 else xt4_next.pop(0)

                pst = pou_pst.setdefault(t, {})

                def make_pou(tt, xt_tt, pstd):
                    def pou_l0():
                        ps0 = pp.tile([PH, PTILE], f32, tag="mm")
                        for rr in range(NCH):
                            nc.tensor.matmul(
                                ps0[:, rr * CHK : (rr + 1) * CHK],
                                s_pw0q[32 * rr : 32 * rr + 3, :],
                                xt_tt[32 * rr : 32 * rr + 3, rr * CHK : (rr + 1) * CHK],
                                start=True,
                                stop=True,
                                tile_position=(32 * rr, 0),
                            )
                        h = hp.tile([PH, PTILE], f32r, tag="h")
                        nc.vector.tensor_scalar_max(h[:], ps0[:], 0.0)
                        pstd["h"] = h

                    def pou_mid(l):
                        h = pstd["h"]
                        psl = pp.tile([PH, PTILE], f32, tag="mm")
                        for n in range(NCH):
                            nc.tensor.matmul(
                                psl[:, n * CHK : (n + 1) * CHK],
                                s_pwm[:, l * PH : (l + 1) * PH],
                                h[:, n * CHK : (n + 1) * CHK],
                                start=True,
                                stop=True,
                            )
                        hr = hp.tile([PH, PTILE], f32r, tag="h")
                        nc.vector.tensor_scalar(
                            hr[:], psl[:], s_pbm[:, l : l + 1], 0.0, op0=OP.add, op1=OP.max
                        )
                        nc.vector.tensor_add(hr[:], hr[:], h[:])
                        pstd["h"] = hr

                    def pou_logits():
                        h = pstd["h"]
                        psz = pp.tile([C, PTILE], f32, tag="mm")
                        for n in range(NCH):
                            nc.tensor.matmul(
                                psz[:, n * CHK : (n + 1) * CHK],
                                s_pwl[:],
                                h[:, n * CHK : (n + 1) * CHK],
                                start=True,
                                stop=True,
                            )
                        ez = sp.tile([C, PTILE], f32r, tag="ez")
                        nc.scalar.activation(ez[:], psz[:], AF.Exp, bias=s_pbl[:, 0:1])
                        pstd["ez"] = ez

                    def pou_s12():
                        ez = pstd["ez"]
                        pss = pp.tile([2, PTILE], f32, tag="mm")
                        for n in range(NCH):
                            nc.tensor.matmul(
                                pss[:, n * CHK : (n + 1) * CHK],
                                s_cw[:],
                                ez[:, n * CHK : (n + 1) * CHK],
                                start=True,
                                stop=True,
                            )
                        s12row = rp.tile([2, PTILE], f32, tag="row")
                        nc.vector.tensor_copy(s12row[:], pss[:])
                        nc.sync.dma_start(
                            s_s1P[tt * 32 : (tt + 1) * 32, :], s12row[0:1, :]
                        )
                        nc.sync.dma_start(
                            s_s2P[tt * 32 : (tt + 1) * 32, :], s12row[1:2, :]
                        )

                    return [
                        pou_l0,
                        lambda: pou_mid(0),
                        lambda: pou_mid(1),
                        pou_logits,
                        pou_s12,
                    ]

                pou_steps = make_pou(t, xt4, pst) if t == 0 else None

                # ---------- subnets (two staggered lanes) ----------
                u_asm = sp.tile([C, PTILE], f32r, tag="ua")

                def sub_l0(c, st, fc):
                    if fc == 0:
                        st["g0"] = gp.tile([128, KC, PTILE], bf16, tag="g", name="g0")
                    col = (c * FC + fc) * 128
                    pt = pp.tile([128, PTILE], f32, tag="mm")
                    for rr in range(NCH):
                        nc.tensor.matmul(
                            pt[:, rr * CHK : (rr + 1) * CHK],
                            s_w0q[32 * rr : 32 * rr + 3, col : col + 128],
                            xt4[32 * rr : 32 * rr + 3, rr * CHK : (rr + 1) * CHK],
                            start=True,
                            stop=True,
                            tile_position=(32 * rr, 0),
                        )
                    nc.scalar.activation(st["g0"][:, fc, :], pt[:], AF.Tanh)

                def sub_mid(c, st, l, fc):
                    gcur = st[f"g{l}"]
                    if fc == 0:
                        st[f"g{l + 1}"] = gp.tile(
                            [128, KC, PTILE], bf16, tag="g", name=f"g{l + 1}"
                        )
                    pt = pp.tile([128, PTILE], f32, tag="mm")
                    for kc in range(KC):
                        col = c * WBLK + ((l * FC + fc) * KC + kc) * 128
                        for n in range(NCH):
                            nc.tensor.matmul(
                                pt[:, n * CHK : (n + 1) * CHK],
                                s_wm[:, col : col + 128],
                                gcur[:, kc, n * CHK : (n + 1) * CHK],
                                start=(kc == 0),
                                stop=(kc == KC - 1),
                            )
                    bcol = (c * SNMID + l) * FC + fc
                    nc.scalar.activation(
                        st[f"g{l + 1}"][:, fc, :],
                        pt[:],
                        AF.Tanh,
                        bias=s_bm[:, bcol : bcol + 1],
                    )

                def pod_last(p):
                    # last layer for subnets 4p..4p+3, one [1,512] output per
                    # (subnet, chunk) packed into 4 concurrent PE col groups
                    pu = pp.tile([128, PTILE], f32, tag="mm")
                    for kc in range(KC):
                        for n in range(NCH):
                            for j in range(4):
                                c = 4 * p + j
                                gcur = states[c][f"g{SNMID}"]
                                nc.tensor.matmul(
                                    pu[32 * j : 32 * j + 1, n * CHK : (n + 1) * CHK],
                                    s_wl[:, c * KC + kc : c * KC + kc + 1],
                                    gcur[:, kc, n * CHK : (n + 1) * CHK],
                                    start=(kc == 0),
                                    stop=(kc == KC - 1),
                                    tile_position=(0, 32 * j),
                                )
                    urows = rp.tile([128, PTILE], f32r, tag="row")
                    nc.vector.tensor_copy(urows[0:97, :], pu[0:97, :])
                    for j in range(4):
                        nc.sync.dma_start(
                            u_asm[4 * p + j : 4 * p + j + 1, :],
                            urows[32 * j : 32 * j + 1, :],
                        )

                def pod_pair(ca, cb):
                    # 2-subnet last-layer piece (last tile's tail: most of
                    # the final pod's fill overlaps remaining ACT work)
                    pu = pp.tile([128, PTILE], f32, tag="mm")
                    for kc in range(KC):
                        for n in range(NCH):
                            for j, c in enumerate((ca, cb)):
                                gcur = states[c][f"g{SNMID}"]
                                nc.tensor.matmul(
                                    pu[32 * j : 32 * j + 1, n * CHK : (n + 1) * CHK],
                                    s_wl[:, c * KC + kc : c * KC + kc + 1],
                                    gcur[:, kc, n * CHK : (n + 1) * CHK],
                                    start=(kc == 0),
                                    stop=(kc == KC - 1),
                                    tile_position=(0, 32 * j),
                                )
                    urows = rp.tile([128, PTILE], f32r, tag="row")
                    nc.vector.tensor_copy(urows[0:33, :], pu[0:33, :])
                    for j, c in enumerate((ca, cb)):
                        nc.sync.dma_start(
                            u_asm[c : c + 1, :], urows[32 * j : 32 * j + 1, :]
                        )

                def sub_step(c, st, k):
                    if k <= 1:
                        sub_l0(c, st, k)
                    else:
                        sub_mid(c, st, (k - 2) // FC, (k - 2) % FC)

                # lane A: even subnets, lane B: odd subnets (B lags by STAG);
                # chains run 6 steps (l0 x2, mid x4); last layer runs as pods
                # of 4 subnets packed into PE col quadrants
                stepsA = [(c, k) for c in range(0, C, 2) for k in range(6)]
                stepsB = [(c, k) for c in range(1, C, 2) for k in range(6)]
                states = [dict() for _ in range(C)]
                nslots = len(stepsA) + STAG
                pou_next = [None]
                for s in range(nslots):
                    if t == 0 and s < 5:
                        pou_steps[s]()
                    if s == 4 and pending[0] is not None:
                        pending[0]()
                        pending[0] = None
                    if s == 14 and t + 1 < NT:
                        nxt = xpl.tile([128, PTILE], bf16, tag="xt", name="xt4")
                        nc.sync.dma_start(
                            nxt[:], xT[:, (t + 1) * PTILE : (t + 2) * PTILE]
                        )
                        xt4_next[0] = nxt
                        pou_next[0] = make_pou(
                            t + 1, nxt, pou_pst.setdefault(t + 1, {})
                        )
                    if 18 <= s <= 30 and (s - 18) % 3 == 0 and pou_next[0] is not None:
                        pou_next[0][(s - 18) // 3]()
                    if s < len(stepsA):
                        c, k = stepsA[s]
                        sub_step(c, states[c], k)
                    if s >= STAG:
                        c, k = stepsB[s - STAG]
                        sub_step(c, states[c], k)
                    if s >= 14 and (s - 14) % 12 == 0:
                        if s == 50 and t == NT - 1:
                            pod_pair(14, 15)
                        else:
                            pod_last((s - 14) // 12)
                    if s == 45 and t == NT - 1:
                        pod_pair(12, 13)

                # ---------- windowed combine (deferred into next tile) ----------
                def make_combine(t, u_asm, ez):
                    def combine():
                        # DVE mul on the last tile: it sits on the serial tail
                        if t == NT - 1:
                            nc.vector.tensor_mul(u_asm[:], u_asm[:], ez[:])
                        else:
                            nc.gpsimd.tensor_mul(u_asm[:], u_asm[:], ez[:])
                        pacc = pp.tile([1, PTILE], f32, tag="mm")
                        for n in range(NCH):
                            nc.tensor.matmul(
                                pacc[:, n * CHK : (n + 1) * CHK],
                                s_cw[:, 0:1],
                                u_asm[:, n * CHK : (n + 1) * CHK],
                                start=True,
                                stop=True,
                            )
                        accrow = rp.tile([2, PTILE], f32, tag="row")
                        nc.vector.tensor_copy(accrow[0:1, :], pacc[:])
                        nc.sync.dma_start(
                            s_accP[t * 32 : (t + 1) * 32, :], accrow[0:1, :]
                        )

                    return combine

                pending[0] = make_combine(t, u_asm, pst["ez"])

            if pending[0] is not None:
                pending[0]()
                pending[0] = None

            # ---------- final: combine in points-layout ----------
            s_r = fin.tile([128, PPB], f32)
            nc.vector.reciprocal(s_r[:], s_s1P[:])
            s_num = fin.tile([128, PPB], f32)
            nc.vector.tensor_add(s_num[:], s_accP[:], s_s2P[:])
            nc.vector.tensor_mul(s_num[:], s_num[:], s_r[:])
            nc.vector.tensor_mul(s_num[:], s_num[:], s_bc[:])
            nc.sync.dma_start(y.rearrange("(p j) -> p j", p=128), s_num[:])

    nc.compile()
    return nc


def _prep_inputs(inputs):
    import ml_dtypes

    f = lambda k: np.ascontiguousarray(np.asarray(inputs[k]), dtype=np.float32)
    x = f("x")
    centers, scales = f("centers"), f("scales")
    sub_W0, sub_b0 = f("sub_W0"), f("sub_b0")
    sub_Wmid, sub_bmid = f("sub_Wmid"), f("sub_bmid")
    sub_Wl, sub_bl = f("sub_Wl"), f("sub_bl")

    # fold per-subdomain normalization into layer-0 weights:
    # xn = (x - c)/s  =>  xn @ W0 + b0 = x @ (W0/s) + (b0 - (c/s) @ W0)
    w0e_full = sub_W0 / scales[:, :, None]                       # [C, D, SW]
    b0e_full = sub_b0 - np.einsum("cd,cdw->cw", centers / scales, sub_W0)

    # row-group packed layer-0 weights: rows {32r,32r+1,32r+2} = [W0; W1; b]
    w0q = np.zeros((128, C * FC * 128), np.float32)
    for c in range(C):
        for fc in range(FC):
            col = (c * FC + fc) * 128
            blk = np.vstack(
                [
                    w0e_full[c][:, fc * 128 : (fc + 1) * 128],
                    b0e_full[c][None, fc * 128 : (fc + 1) * 128],
                ]
            )
            for rr in range(4):
                w0q[32 * rr : 32 * rr + 3, col : col + 128] = blk

    middt_np = ml_dtypes.bfloat16
    wm = np.ascontiguousarray(
        sub_Wmid.reshape(C, SNMID, KC, 128, FC, 128)
        .transpose(3, 0, 1, 4, 2, 5)
        .reshape(128, C * WBLK)
    ).astype(middt_np)
    bm = np.ascontiguousarray(
        sub_bmid.reshape(C, SNMID, FC, 128).transpose(3, 0, 1, 2).reshape(128, -1)
    )
    wl = np.ascontiguousarray(
        sub_Wl.reshape(C, KC, 128).transpose(2, 0, 1).reshape(128, -1)
    ).astype(middt_np)
    cwm = np.ascontiguousarray(
        np.stack([np.ones(C, np.float32), sub_bl[:, 0]], axis=1)
    )

    pw0q = np.zeros((128, PH), np.float32)
    pblk = np.vstack([f("pou_W0"), f("pou_b0")[None, :]])
    for rr in range(4):
        pw0q[32 * rr : 32 * rr + 3, :] = pblk

    shared = dict(
        w0q=w0q.astype(middt_np),
        wm=wm,
        bm=bm,
        wl=wl,
        pw0q=pw0q.astype(middt_np),
        pwm=np.ascontiguousarray(f("pou_Wmid").transpose(1, 0, 2).reshape(PH, -1)),
        pbm=np.ascontiguousarray(f("pou_bmid").T),
        pwl=f("pou_Wl"),
        pbl=np.ascontiguousarray(f("pou_bl")[:, None]),
        cw=cwm,
    )

    in_maps = []
    for core in range(NCORES):
        xs = x[core * NP : (core + 1) * NP]
        m = dict(shared)
        xt3 = np.vstack([xs.T, np.ones((1, NP), np.float32)])  # [3, NP]
        xtq = np.zeros((128, NP), np.float32)
        for rr in range(4):
            xtq[32 * rr : 32 * rr + 3, :] = xt3
        m["xT"] = np.ascontiguousarray(xtq).astype(ml_dtypes.bfloat16)
        m["xP"] = np.ascontiguousarray(xs.reshape(128, 2 * PPB))
        in_maps.append(m)
    return in_maps


def kernel(**inputs):
    from concourse.bass_utils import run_bass_kernel_spmd

    if "nc" not in _CACHE:
        _CACHE["nc"] = _build()
    nc = _CACHE["nc"]

    in_maps = _prep_inputs(inputs)
    trace = os.environ.get("KERNEL_TRACE", "0") == "1"
    res = run_bass_kernel_spmd(
        nc, in_maps, core_ids=list(range(NCORES)), trace=trace
    )
    kernel.last_results = res
    y = np.concatenate([res.results[i]["y"] for i in range(NCORES)])
    return y.astype(np.float32)


# revision 44
# speedup vs baseline: 1.0106x; 1.0038x over previous
"""Trainium2 Bass kernel for FBPINN-with-window (dense MoE over 16 subnets).

Math (per point n):
    h   = relu(x @ pW0 + pb0); h += relu(h @ pWmid_l + pbmid_l) (x2)
    z   = h @ pWl + pbl;  ez = exp(z)            (softmax un-normalized)
    xn_c = (x - center_c)/scale_c  (folded on host into layer-0 weights)
    g_c = tanh(xn_c @ W0_c + b0_c); g_c = tanh(g_c @ Wmid_cl + bmid_cl) (x2)
    u_c = g_c @ Wl_c + bl_c
    acc = sum_c softmax(z)_c * u_c = (sum_c ez_c*(g_c@Wl_c) + sum_c ez_c*bl_c) / sum_c ez_c
    out = acc * x0(1-x0)*x1(1-x1)

Device layout: activations transposed (features on partitions, points on the
free dim).  Data-parallel over 8 cores (8192 points each), 4 point-tiles of
2048 per core.  The ACT (scalar) engine's tanh throughput is the roofline
(~96 x [128,2048] tanh per tile); the design goal is keeping ACT ~100% busy:
  - all subnet weights resident in SBUF (bf16 for mids/last), no streaming
  - two subnet chains in flight, staggered by 3 steps, so one chain's
    matmuls/psum drains overlap the other chain's tanh
  - per-chain last-layer output (PSUM) drained by DVE + DMA off ACT's path
  - x staged in DRAM pre-replicated at rows 32r (one contiguous DMA per
    point-tile); prologue DMAs split between the sync queue (critical path:
    tile-0 x, first subnets' weights) and the gpsimd queue (bulk weights)
"""

import os

import numpy as np

N = 65536
D = 2
C = 16
PH = 128
PNMID = 2
SW = 256
SNMID = 2

NCORES = 8
NP = N // NCORES          # 8192 points per core
PTILE = 2048              # points per tile
NT = NP // PTILE          # 4 tiles
CHK = 512                 # matmul moving free dim (one PSUM bank)
NCH = PTILE // CHK        # 4 chunks per tile
FC = SW // 128            # 2 feature chunks
KC = SW // 128            # 2 contraction chunks
PPB = NP // 128           # 64 points per partition (points-layout)
WBLK = SNMID * FC * KC * 128  # mid-weight cols per subnet (1024)

STAG = int(os.environ.get("KERNEL_STAG", "3"))  # lane-B stagger in steps

_CACHE = {}


def _build():
    import concourse.mybir as mybir
    import concourse.tile as tile
    from concourse import bacc

    f32 = mybir.dt.float32
    f32r = mybir.dt.float32r
    bf16 = mybir.dt.bfloat16
    AF = mybir.ActivationFunctionType
    OP = mybir.AluOpType

    nc = bacc.Bacc("TRN2", debug=False)

    def din(name, shape, dt=f32):
        return nc.dram_tensor(name, shape, dt, kind="ExternalInput").ap()

    xT = din("xT", (128, NP), bf16)   # [x0;x1;1] replicated at rows 32r
    xP = din("xP", (128, 2 * PPB))
    w0q = din("w0q", (128, C * FC * 128), bf16)
    wm = din("wm", (128, C * WBLK), bf16)
    bm = din("bm", (128, C * SNMID * FC))
    wl = din("wl", (128, C * KC), bf16)
    pw0q = din("pw0q", (128, PH), bf16)
    pwm = din("pwm", (PH, PNMID * PH), f32r)
    pbm = din("pbm", (PH, PNMID))
    pwl = din("pwl", (PH, C), f32r)
    pbl = din("pbl", (C, 1))
    cw = din("cw", (C, 2), f32r)
    y = nc.dram_tensor("y", (NP,), f32, kind="ExternalOutput").ap()

    with tile.TileContext(nc) as tc:
        with (
            tc.tile_pool(name="wp", bufs=1) as wp,
            tc.tile_pool(name="gp", bufs=10) as gp,
            tc.tile_pool(name="hp", bufs=2) as hp,
            tc.tile_pool(name="sp", bufs=2) as sp,
            tc.tile_pool(name="rp", bufs=2) as rp,
            tc.tile_pool(name="xp", bufs=2) as xpl,
            tc.tile_pool(name="fin", bufs=1) as fin,
            tc.tile_pool(name="pp", bufs=2, space="PSUM") as pp,
        ):
            # ---- critical-path DMAs fanned across idle engine queues ----
            xt0 = xpl.tile([128, PTILE], bf16, tag="xt")
            nc.sync.dma_start(xt0[:], xT[:, 0:PTILE])
            s_w0q = wp.tile([128, C * FC * 128], bf16)
            qb = C * FC * 32  # 1024 cols = 4 subnets worth
            nc.scalar.dma_start(s_w0q[:, 0:qb], w0q[:, 0:qb])
            s_pw0q = wp.tile([128, PH], bf16)
            nc.scalar.dma_start(s_pw0q[:], pw0q)
            s_wm = wp.tile([128, C * WBLK], bf16)
            wb = C * WBLK // 8  # 2048 cols = 2 subnets worth
            nc.gpsimd.dma_start(s_wm[:, 0:wb], wm[:, 0:wb])
            s_bm = wp.tile([128, C * SNMID * FC], f32)
            nc.sync.dma_start(s_bm[:], bm)
            s_pwm = wp.tile([PH, PNMID * PH], f32r)
            nc.sync.dma_start(s_pwm[:], pwm)
            # ---- bulk weights on the (idle) gpsimd queue ----
            s_pbm = wp.tile([PH, PNMID], f32)
            nc.gpsimd.dma_start(s_pbm[:], pbm)
            s_pwl = wp.tile([PH, C], f32r)
            nc.gpsimd.dma_start(s_pwl[:], pwl)
            s_pbl = wp.tile([C, 1], f32)
            nc.gpsimd.dma_start(s_pbl[:], pbl)
            for cq in range(1, 4):
                nc.gpsimd.dma_start(
                    s_w0q[:, cq * qb : (cq + 1) * qb], w0q[:, cq * qb : (cq + 1) * qb]
                )
            s_wl = wp.tile([128, C * KC], bf16)
            nc.gpsimd.dma_start(s_wl[:], wl)
            s_cw = wp.tile([C, 2], f32r)
            nc.gpsimd.dma_start(s_cw[:], cw)
            for cq in range(1, 8):
                nc.gpsimd.dma_start(
                    s_wm[:, cq * wb : (cq + 1) * wb], wm[:, cq * wb : (cq + 1) * wb]
                )

            # ---- per-core x (points-layout) + boundary factor ----
            s_xP = fin.tile([128, 2 * PPB], f32)
            nc.gpsimd.dma_start(s_xP[:], xP)
            s_xmx = fin.tile([128, 2 * PPB], f32)
            nc.vector.tensor_mul(s_xmx[:], s_xP[:], s_xP[:])
            nc.vector.tensor_sub(s_xmx[:], s_xP[:], s_xmx[:])
            v = s_xmx.rearrange("p (j two) -> p j two", two=2)
            s_bc = fin.tile([128, PPB], f32)
            nc.vector.tensor_mul(s_bc[:], v[:, :, 0], v[:, :, 1])

            # points-layout accumulators, filled per tile via reshape DMAs
            s_accP = fin.tile([128, PPB], f32)
            s_s1P = fin.tile([128, PPB], f32)
            s_s2P = fin.tile([128, PPB], f32)

            xt4_next = {}
            pou_pst = {}
            pending = [None]
            for t in range(NT):
                toff = t * PTILE
                xt4 = xt0 if t == 0 else xt4_next.pop(0)

                pst = pou_pst.setdefault(t, {})

                def make_pou(tt, xt_tt, pstd):
                    def pou_l0():
                        ps0 = pp.tile([PH, PTILE], f32, tag="mm")
                        for rr in range(NCH):
                            nc.tensor.matmul(
                                ps0[:, rr * CHK : (rr + 1) * CHK],
                                s_pw0q[32 * rr : 32 * rr + 3, :],
                                xt_tt[32 * rr : 32 * rr + 3, rr * CHK : (rr + 1) * CHK],
                                start=True,
                                stop=True,
                                tile_position=(32 * rr, 0),
                            )
                        h = hp.tile([PH, PTILE], f32r, tag="h")
                        nc.vector.tensor_scalar_max(h[:], ps0[:], 0.0)
                        pstd["h"] = h

                    def pou_mid(l):
                        h = pstd["h"]
                        psl = pp.tile([PH, PTILE], f32, tag="mm")
                        for n in range(NCH):
                            nc.tensor.matmul(
                                psl[:, n * CHK : (n + 1) * CHK],
                                s_pwm[:, l * PH : (l + 1) * PH],
                                h[:, n * CHK : (n + 1) * CHK],
                                start=True,
                                stop=True,
                            )
                        hr = hp.tile([PH, PTILE], f32r, tag="h")
                        nc.vector.tensor_scalar(
                            hr[:], psl[:], s_pbm[:, l : l + 1], 0.0, op0=OP.add, op1=OP.max
                        )
                        nc.vector.tensor_add(hr[:], hr[:], h[:])
                        pstd["h"] = hr

                    def pou_logits():
                        h = pstd["h"]
                        psz = pp.tile([C, PTILE], f32, tag="mm")
                        for n in range(NCH):
                            nc.tensor.matmul(
                                psz[:, n * CHK : (n + 1) * CHK],
                                s_pwl[:],
                                h[:, n * CHK : (n + 1) * CHK],
                                start=True,
                                stop=True,
                            )
                        ez = sp.tile([C, PTILE], f32r, tag="ez")
                        nc.scalar.activation(ez[:], psz[:], AF.Exp, bias=s_pbl[:, 0:1])
                        pstd["ez"] = ez

                    def pou_s12():
                        ez = pstd["ez"]
                        pss = pp.tile([2, PTILE], f32, tag="mm")
                        for n in range(NCH):
                            nc.tensor.matmul(
                                pss[:, n * CHK : (n + 1) * CHK],
                                s_cw[:],
                                ez[:, n * CHK : (n + 1) * CHK],
                                start=True,
                                stop=True,
                            )
                        s12row = rp.tile([2, PTILE], f32, tag="row")
                        nc.vector.tensor_copy(s12row[:], pss[:])
                        nc.sync.dma_start(
                            s_s1P[tt * 32 : (tt + 1) * 32, :], s12row[0:1, :]
                        )
                        nc.sync.dma_start(
                            s_s2P[tt * 32 : (tt + 1) * 32, :], s12row[1:2, :]
                        )

                    return [
                        pou_l0,
                        lambda: pou_mid(0),
                        lambda: pou_mid(1),
                        pou_logits,
                        pou_s12,
                    ]

                pou_steps = make_pou(t, xt4, pst) if t == 0 else None

                # ---------- subnets (two staggered lanes) ----------
                u_asm = sp.tile([C, PTILE], f32r, tag="ua")

                def sub_l0(c, st, fc):
                    if fc == 0:
                        st["g0"] = gp.tile([128, KC, PTILE], bf16, tag="g", name="g0")
                    col = (c * FC + fc) * 128
                    pt = pp.tile([128, PTILE], f32, tag="mm")
                    for rr in range(NCH):
                        nc.tensor.matmul(
                            pt[:, rr * CHK : (rr + 1) * CHK],
                            s_w0q[32 * rr : 32 * rr + 3, col : col + 128],
                            xt4[32 * rr : 32 * rr + 3, rr * CHK : (rr + 1) * CHK],
                            start=True,
                            stop=True,
                            tile_position=(32 * rr, 0),
                        )
                    nc.scalar.activation(st["g0"][:, fc, :], pt[:], AF.Tanh)

                def sub_mid(c, st, l, fc):
                    gcur = st[f"g{l}"]
                    if fc == 0:
                        st[f"g{l + 1}"] = gp.tile(
                            [128, KC, PTILE], bf16, tag="g", name=f"g{l + 1}"
                        )
                    pt = pp.tile([128, PTILE], f32, tag="mm")
                    for kc in range(KC):
                        col = c * WBLK + ((l * FC + fc) * KC + kc) * 128
                        for n in range(NCH):
                            nc.tensor.matmul(
                                pt[:, n * CHK : (n + 1) * CHK],
                                s_wm[:, col : col + 128],
                                gcur[:, kc, n * CHK : (n + 1) * CHK],
                                start=(kc == 0),
                                stop=(kc == KC - 1),
                            )
                    bcol = (c * SNMID + l) * FC + fc
                    nc.scalar.activation(
                        st[f"g{l + 1}"][:, fc, :],
                        pt[:],
                        AF.Tanh,
                        bias=s_bm[:, bcol : bcol + 1],
                    )

                def pod_last(p):
                    # last layer for subnets 4p..4p+3, one [1,512] output per
                    # (subnet, chunk) packed into 4 concurrent PE col groups
                    pu = pp.tile([128, PTILE], f32, tag="mm")
                    for kc in range(KC):
                        for n in range(NCH):
                            for j in range(4):
                                c = 4 * p + j
                                gcur = states[c][f"g{SNMID}"]
                                nc.tensor.matmul(
                                    pu[32 * j : 32 * j + 1, n * CHK : (n + 1) * CHK],
                                    s_wl[:, c * KC + kc : c * KC + kc + 1],
                                    gcur[:, kc, n * CHK : (n + 1) * CHK],
                                    start=(kc == 0),
                                    stop=(kc == KC - 1),
                                    tile_position=(0, 32 * j),
                                )
                    urows = rp.tile([128, PTILE], f32r, tag="row")
                    nc.vector.tensor_copy(urows[0:97, :], pu[0:97, :])
                    for j in range(4):
                        nc.sync.dma_start(
                            u_asm[4 * p + j : 4 * p + j + 1, :],
                            urows[32 * j : 32 * j + 1, :],
                        )

                def pod_pair(ca, cb):
                    # 2-subnet last-layer piece (last tile's tail: most of
                    # the final pod's fill overlaps remaining ACT work)
                    pu = pp.tile([128, PTILE], f32, tag="mm")
                    for kc in range(KC):
                        for n in range(NCH):
                            for j, c in enumerate((ca, cb)):
                                gcur = states[c][f"g{SNMID}"]
                                nc.tensor.matmul(
                                    pu[32 * j : 32 * j + 1, n * CHK : (n + 1) * CHK],
                                    s_wl[:, c * KC + kc : c * KC + kc + 1],
                                    gcur[:, kc, n * CHK : (n + 1) * CHK],
                                    start=(kc == 0),
                                    stop=(kc == KC - 1),
                                    tile_position=(0, 32 * j),
                                )
                    urows = rp.tile([128, PTILE], f32r, tag="row")
                    nc.vector.tensor_copy(urows[0:33, :], pu[0:33, :])
                    for j, c in enumerate((ca, cb)):
                        nc.sync.dma_start(
                            u_asm[c : c + 1, :], urows[32 * j : 32 * j + 1, :]
                        )

                def sub_step(c, st, k):
                    if k <= 1:
                        sub_l0(c, st, k)
                    else:
                        sub_mid(c, st, (k - 2) // FC, (k - 2) % FC)

                # lane A: even subnets, lane B: odd subnets (B lags by STAG);
                # chains run 6 steps (l0 x2, mid x4); last layer runs as pods
                # of 4 subnets packed into PE col quadrants
                stepsA = [(c, k) for c in range(0, C, 2) for k in range(6)]
                stepsB = [(c, k) for c in range(1, C, 2) for k in range(6)]
                states = [dict() for _ in range(C)]
                nslots = len(stepsA) + STAG
                pou_next = [None]
                T0_POU = {5: 0, 8: 1, 11: 2, 13: 3, 16: 4}
                for s in range(nslots):
                    # tile-0 pou runs after the first subnets' tiles are in
                    # flight (head psum slots are contended while weight DMAs
                    # land); its only deadline is combine(0) at tile-1 slot 4
                    if t == 0 and s in T0_POU:
                        pou_steps[T0_POU[s]]()
                    if s == 4 and pending[0] is not None:
                        pending[0]()
                        pending[0] = None
                    if s == 14 and t + 1 < NT:
                        nxt = xpl.tile([128, PTILE], bf16, tag="xt", name="xt4")
                        nc.sync.dma_start(
                            nxt[:], xT[:, (t + 1) * PTILE : (t + 2) * PTILE]
                        )
                        xt4_next[0] = nxt
                        pou_next[0] = make_pou(
                            t + 1, nxt, pou_pst.setdefault(t + 1, {})
                        )
                    if 18 <= s <= 30 and (s - 18) % 3 == 0 and pou_next[0] is not None:
                        pou_next[0][(s - 18) // 3]()
                    if s < len(stepsA):
                        c, k = stepsA[s]
                        sub_step(c, states[c], k)
                    if s >= STAG:
                        c, k = stepsB[s - STAG]
                        sub_step(c, states[c], k)
                    if s >= 14 and (s - 14) % 12 == 0:
                        if s == 50 and t == NT - 1:
                            pod_pair(14, 15)
                        else:
                            pod_last((s - 14) // 12)
                    if s == 45 and t == NT - 1:
                        pod_pair(12, 13)

                # ---------- windowed combine (deferred into next tile) ----------
                def make_combine(t, u_asm, ez):
                    def combine():
                        # DVE mul on the last tile: it sits on the serial tail
                        if t == NT - 1:
                            nc.vector.tensor_mul(u_asm[:], u_asm[:], ez[:])
                        else:
                            nc.gpsimd.tensor_mul(u_asm[:], u_asm[:], ez[:])
                        pacc = pp.tile([1, PTILE], f32, tag="mm")
                        for n in range(NCH):
                            nc.tensor.matmul(
                                pacc[:, n * CHK : (n + 1) * CHK],
                                s_cw[:, 0:1],
                                u_asm[:, n * CHK : (n + 1) * CHK],
                                start=True,
                                stop=True,
                            )
                        accrow = rp.tile([2, PTILE], f32, tag="row")
                        nc.vector.tensor_copy(accrow[0:1, :], pacc[:])
                        nc.sync.dma_start(
                            s_accP[t * 32 : (t + 1) * 32, :], accrow[0:1, :]
                        )

                    return combine

                pending[0] = make_combine(t, u_asm, pst["ez"])

            if pending[0] is not None:
                pending[0]()
                pending[0] = None

            # ---------- final: combine in points-layout ----------
            s_r = fin.tile([128, PPB], f32)
            nc.vector.reciprocal(s_r[:], s_s1P[:])
            s_num = fin.tile([128, PPB], f32)
            nc.vector.tensor_add(s_num[:], s_accP[:], s_s2P[:])
            nc.vector.tensor_mul(s_num[:], s_num[:], s_r[:])
            nc.vector.tensor_mul(s_num[:], s_num[:], s_bc[:])
            nc.sync.dma_start(y.rearrange("(p j) -> p j", p=128), s_num[:])

    nc.compile()
    return nc


def _prep_inputs(inputs):
    import ml_dtypes

    f = lambda k: np.ascontiguousarray(np.asarray(inputs[k]), dtype=np.float32)
    x = f("x")
    centers, scales = f("centers"), f("scales")
    sub_W0, sub_b0 = f("sub_W0"), f("sub_b0")
    sub_Wmid, sub_bmid = f("sub_Wmid"), f("sub_bmid")
    sub_Wl, sub_bl = f("sub_Wl"), f("sub_bl")

    # fold per-subdomain normalization into layer-0 weights:
    # xn = (x - c)/s  =>  xn @ W0 + b0 = x @ (W0/s) + (b0 - (c/s) @ W0)
    w0e_full = sub_W0 / scales[:, :, None]                       # [C, D, SW]
    b0e_full = sub_b0 - np.einsum("cd,cdw->cw", centers / scales, sub_W0)

    # row-group packed layer-0 weights: rows {32r,32r+1,32r+2} = [W0; W1; b]
    w0q = np.zeros((128, C * FC * 128), np.float32)
    for c in range(C):
        for fc in range(FC):
            col = (c * FC + fc) * 128
            blk = np.vstack(
                [
                    w0e_full[c][:, fc * 128 : (fc + 1) * 128],
                    b0e_full[c][None, fc * 128 : (fc + 1) * 128],
                ]
            )
            for rr in range(4):
                w0q[32 * rr : 32 * rr + 3, col : col + 128] = blk

    middt_np = ml_dtypes.bfloat16
    wm = np.ascontiguousarray(
        sub_Wmid.reshape(C, SNMID, KC, 128, FC, 128)
        .transpose(3, 0, 1, 4, 2, 5)
        .reshape(128, C * WBLK)
    ).astype(middt_np)
    bm = np.ascontiguousarray(
        sub_bmid.reshape(C, SNMID, FC, 128).transpose(3, 0, 1, 2).reshape(128, -1)
    )
    wl = np.ascontiguousarray(
        sub_Wl.reshape(C, KC, 128).transpose(2, 0, 1).reshape(128, -1)
    ).astype(middt_np)
    cwm = np.ascontiguousarray(
        np.stack([np.ones(C, np.float32), sub_bl[:, 0]], axis=1)
    )

    pw0q = np.zeros((128, PH), np.float32)
    pblk = np.vstack([f("pou_W0"), f("pou_b0")[None, :]])
    for rr in range(4):
        pw0q[32 * rr : 32 * rr + 3, :] = pblk

    shared = dict(
        w0q=w0q.astype(middt_np),
        wm=wm,
        bm=bm,
        wl=wl,
        pw0q=pw0q.astype(middt_np),
        pwm=np.ascontiguousarray(f("pou_Wmid").transpose(1, 0, 2).reshape(PH, -1)),
        pbm=np.ascontiguousarray(f("pou_bmid").T),
        pwl=f("pou_Wl"),
        pbl=np.ascontiguousarray(f("pou_bl")[:, None]),
        cw=cwm,
    )

    in_maps = []
    for core in range(NCORES):
        xs = x[core * NP : (core + 1) * NP]
        m = dict(shared)
        xt3 = np.vstack([xs.T, np.ones((1, NP), np.float32)])  # [3, NP]
        xtq = np.zeros((128, NP), np.float32)
        for rr in range(4):
            xtq[32 * rr : 32 * rr + 3, :] = xt3
        m["xT"] = np.ascontiguousarray(xtq).astype(ml_dtypes.bfloat16)
        m["xP"] = np.ascontiguousarray(xs.reshape(128, 2 * PPB))
        in_maps.append(m)
    return in_maps


def kernel(**inputs):
    from concourse.bass_utils import run_bass_kernel_spmd

    if "nc" not in _CACHE:
        _CACHE["nc"] = _build()
    nc = _CACHE["nc"]

    in_maps = _prep_inputs(inputs)
    trace = os.environ.get("KERNEL_TRACE", "0") == "1"
    res = run_bass_kernel_spmd(
        nc, in_maps, core_ids=list(range(NCORES)), trace=trace
    )
    kernel.last_results = res
    y = np.concatenate([res.results[i]["y"] for i in range(NCORES)])
    return y.astype(np.float32)
